# revision 29
# baseline (speedup 1.0000x reference)
"""Trainium2 Bass kernel for nn_CrossAttentionLayer (ragged cross-attention + MLP).

Sharding: 64 ragged segments -> 8 cores x 8 slots. Segments are sorted by
(src-chunk count, dst count), dealt into slots of 8, then hill-climbed so
segments sharing a slot have similar sizes; each slot is trimmed to the max
dst count (ND) / src count (NS, rounded to 128) over its 8 segments, so all
cores run one SPMD program.

All matmul operands are bf16 (fp32 PSUM accumulation). Activations stay
channel-major [chan, tok]; softmax runs in scoresT orientation [src, dst]
with the src-padding mask applied as a per-partition bias on the exp. The
denominator is computed by summing e over src chunks on DVE/GpSimd, then one
banded-ones matmul per head replicates each head's denominator across its 32
partitions; normalization is reciprocal_approx_fast + multiply. V is
produced directly in natural [tok, chan] orientation (src chunks as
stationary); its bias and the merge conv + BN are folded into the MLP
weights on the host. The dst residual is added via an identity matmul
accumulated into the MLP2 PSUM group.

All weights/biases/masks ship in one packed bf16 DMA (f32 parts bitcast);
per-slot inputs/outputs move as single 3D-AP DMAs with a 3-buffer rotation
and 2-slot prefetch.
"""
import math
import sys
from contextlib import ExitStack

import numpy as np
import ml_dtypes

try:
    import concourse.bass as bass
except ImportError:
    sys.path.insert(0, "/opt/trn_rl_repo")
    import concourse.bass as bass

import concourse.tile as tile
from concourse import bacc, mybir
from concourse.bass_utils import run_bass_kernel_spmd

F32 = mybir.dt.float32
BF16 = mybir.dt.bfloat16
BF = ml_dtypes.bfloat16

H = 256          # h_dim
C = 128          # h_div
HEADS = 4
DH = 32
NCORES = 8
MASK_NEG = -20000.0

# packed-weights column offsets (bf16 elements per partition)
OFF_WQ = 0
OFF_WK = 256
OFF_WV = 512
OFF_W1 = 768
OFF_W2 = 1536
OFF_ONES = 2048
OFF_EYE = 2560
OFF_PB = 2688    # 6 f32 = 12 bf16
OFF_MASK = 2700  # WM f32 = 2*WM bf16

# Filled by host_prep; read by build_nc/in_map (same process).
_PLAN = {}


def _make_plan(nd, ns):
    """Assign 64 segments to (core, slot); compute per-slot widths."""
    B = len(nd)
    slots = B // NCORES
    chunks_of = np.ceil(ns / 128).astype(int)
    order = list(np.lexsort((-nd, -chunks_of)))

    def slot_cost(grp):
        ndm = -(-max(nd[g] for g in grp) // 4) * 4
        chm = max(chunks_of[g] for g in grp)
        return ndm * (14 + 8 * chm + 4) + 2 * 128 * chm + 256 * chm

    rng = np.random.default_rng(0)
    groups = [order[j * NCORES:(j + 1) * NCORES] for j in range(slots)]
    costs = [slot_cost(g) for g in groups]
    for _ in range(30000):
        j1, j2 = rng.integers(0, slots, 2)
        if j1 == j2:
            continue
        i1, i2 = rng.integers(0, NCORES, 2)
        g1, g2 = groups[j1][i1], groups[j2][i2]
        groups[j1][i1], groups[j2][i2] = g2, g1
        c1, c2 = slot_cost(groups[j1]), slot_cost(groups[j2])
        if c1 + c2 < costs[j1] + costs[j2]:
            costs[j1], costs[j2] = c1, c2
        else:
            groups[j1][i1], groups[j2][i2] = g1, g2
    # big slots first so the tail slot is the cheapest
    sidx = sorted(range(slots), key=lambda j: -costs[j])
    groups = [groups[j] for j in sidx]

    assign = np.empty((NCORES, slots), dtype=int)   # (core, slot) -> segment
    ND = np.empty(slots, dtype=int)
    NS = np.empty(slots, dtype=int)
    for j in range(slots):
        grp = groups[j]
        for c in range(NCORES):
            assign[c, j] = grp[c]
        ND[j] = int(-(-max(nd[g] for g in grp) // 4) * 4)       # mult of 4
        NS[j] = int(-(-max(ns[g] for g in grp) // 128) * 128)   # mult of 128
    CH = (NS // 128).astype(int)
    return dict(slots=slots, assign=assign, ND=ND, NS=NS, CH=CH,
                doff=np.concatenate([[0], np.cumsum(ND)[:-1]]),
                soff=np.concatenate([[0], np.cumsum(NS)[:-1]]),
                moff=np.concatenate([[0], np.cumsum(CH)[:-1]]),
                WD=int(ND.sum()), WS=int(NS.sum()), WM=int(CH.sum()))


def host_prep(inputs):
    src_h = np.asarray(inputs['src_h'], np.float32)
    dst_h = np.asarray(inputs['dst_h'], np.float32)
    ns = np.asarray(inputs['src_num_verts']).astype(np.int64)
    nd = np.asarray(inputs['dst_num_verts']).astype(np.int64)
    soff_g = np.concatenate([[0], np.cumsum(ns)[:-1]])
    doff_g = np.concatenate([[0], np.cumsum(nd)[:-1]])

    plan = _make_plan(nd, ns)
    global _PLAN
    _PLAN = plan
    slots = plan['slots']

    perm = np.empty(C, np.int64)
    for chat in range(C):
        h, d = divmod(chat, DH)
        perm[chat] = d * HEADS + h
    s = 1.0 / math.sqrt(DH)

    f32 = lambda k: np.asarray(inputs[k], np.float32)
    Wq, bq = f32('Wq'), f32('bq')
    Wk, bk = f32('Wk'), f32('bk')
    Wv, bv = f32('Wv'), f32('bv')
    Wm, bm = f32('Wm'), f32('bm')
    W1, b1 = f32('W1'), f32('b1')
    W2, b2 = f32('W2'), f32('b2')
    g1, be1, rm1, rv1 = f32('g1'), f32('be1'), f32('rm1'), f32('rv1')
    g2, be2, rm2, rv2 = f32('g2'), f32('be2'), f32('rm2'), f32('rv2')

    WqT = (Wq[perm] * s).T                                # [256,128]
    bq_s = bq[perm] * s
    WkT = Wk[perm].T
    WvT = Wv[perm].T
    bv_r = bv[perm]
    Wm_p = Wm[:, perm]
    a1 = g1 / np.sqrt(rv1 + 1e-5)
    W1_f = W1 * a1[:, None]
    b1_f = b1 * a1 + be1 - rm1 * a1
    a2 = g2 / np.sqrt(rv2 + 1e-5)
    W2_f = W2 * a2[:, None]
    b2_f = b2 * a2 + be2 - rm2 * a2
    W1m_p = W1_f[:, H:] @ Wm_p
    # V bias folded all the way into the MLP1 bias: msg enters MLP1 as
    # msg/den (no bias), and W1m_p @ bv_perm is a constant.
    b1_p = b1_f + W1_f[:, H:] @ bm + W1m_p @ bv_r
    W1T = np.concatenate([W1_f[:, :H], W1m_p], axis=1).T  # [384,256]
    W2T = W2_f.T                                          # [256,256]

    pbias = np.zeros((128, 6), np.float32)
    pbias[:, 0] = bq_s
    pbias[:, 1] = bk[perm]
    pbias[:, 2] = b1_p[:128]
    pbias[:, 3] = b1_p[128:]
    pbias[:, 4] = b2_f[:128]
    pbias[:, 5] = b2_f[128:]

    onespad = np.zeros((128, HEADS, C), BF)
    for h in range(HEADS):
        onespad[:, h, h * DH:(h + 1) * DH] = 1.0

    WM = plan['WM']
    WTOT = OFF_MASK + 2 * WM

    def pack_weights(maskb):
        wpk = np.zeros((128, WTOT), BF)
        wpk[:, OFF_WQ:OFF_WQ + 256] = WqT.reshape(2, 128, 128).transpose(1, 0, 2).reshape(128, 256).astype(BF)
        wpk[:, OFF_WK:OFF_WK + 256] = WkT.reshape(2, 128, 128).transpose(1, 0, 2).reshape(128, 256).astype(BF)
        wpk[:, OFF_WV:OFF_WV + 256] = WvT.reshape(2, 128, 128).transpose(1, 0, 2).reshape(128, 256).astype(BF)
        wpk[:, OFF_W1:OFF_W1 + 768] = W1T.reshape(3, 128, 256).transpose(1, 0, 2).reshape(128, 768).astype(BF)
        wpk[:, OFF_W2:OFF_W2 + 512] = W2T.reshape(2, 128, 256).transpose(1, 0, 2).reshape(128, 512).astype(BF)
        wpk[:, OFF_ONES:OFF_ONES + 512] = onespad.reshape(128, 512)
        wpk[:, OFF_EYE:OFF_EYE + 128] = np.eye(128, dtype=BF)
        wpk[:, OFF_PB:OFF_PB + 12] = pbias.view(BF)
        wpk[:, OFF_MASK:OFF_MASK + 2 * WM] = maskb.view(BF)
        return wpk

    ND, NS, CH = plan['ND'], plan['NS'], plan['CH']
    doff, soff, moff = plan['doff'], plan['soff'], plan['moff']
    # chunk fully valid on every core -> exp needs no mask bias
    full = np.ones(plan['WM'], bool)
    for j in range(slots):
        for jj in range(CH[j]):
            full[moff[j] + jj] = all(
                int(ns[plan['assign'][c, j]]) >= 128 * (jj + 1) for c in range(NCORES))
    plan['maskfull'] = full
    cores = []
    for c in range(NCORES):
        dstT = np.zeros((H, plan['WD']), BF)
        srcT = np.zeros((H, plan['WS']), BF)
        maskb = np.full((128, WM), MASK_NEG, np.float32)
        for j in range(slots):
            g = plan['assign'][c, j]
            dstT[:, doff[j]:doff[j] + nd[g]] = dst_h[doff_g[g]:doff_g[g] + nd[g]].T.astype(BF)
            srcT[:, soff[j]:soff[j] + ns[g]] = src_h[soff_g[g]:soff_g[g] + ns[g]].T.astype(BF)
            for jj in range(CH[j]):
                valid = max(0, min(128, int(ns[g]) - jj * 128))
                maskb[:valid, moff[j] + jj] = 0.0
        cores.append(dict(dstT=dstT, srcT=srcT, wpack=pack_weights(maskb)))

    meta = dict(nd=nd, doff_g=doff_g, plan=plan)
    return cores, meta


def declare_tensors(nc, plan):
    WTOT = OFF_MASK + 2 * plan['WM']
    aps = {}
    aps['dstT'] = nc.dram_tensor("dstT", [H, plan['WD']], BF16, kind="ExternalInput").ap()
    aps['srcT'] = nc.dram_tensor("srcT", [H, plan['WS']], BF16, kind="ExternalInput").ap()
    aps['wpack'] = nc.dram_tensor("wpack", [128, WTOT], BF16, kind="ExternalInput").ap()
    aps['vzero'] = nc.dram_tensor("vzero", [128, 4 * HEADS * C], BF16, kind="ExternalInput").ap()
    aps['outT'] = nc.dram_tensor("outT", [H, plan['WD']], BF16, kind="ExternalOutput").ap()
    return aps


def _dram3(ap, col0, width, total_w):
    """AP over a [256, total_w] dram tensor: [p=row%128, a=row//128, w]."""
    return bass.AP(tensor=ap.tensor, offset=col0,
                   ap=[[total_w, 128], [128 * total_w, 2], [1, width]])


def build_body(ctx: ExitStack, tc: tile.TileContext, aps, plan):
    nc = tc.nc
    slots = plan['slots']
    ND, NS, CH = plan['ND'], plan['NS'], plan['CH']
    doff, soff, moff = plan['doff'], plan['soff'], plan['moff']
    NDmax = int(ND.max())
    NSmax = int(NS.max())
    CHmax = int(CH.max())
    WTOT = OFF_MASK + 2 * plan['WM']

    wp = ctx.enter_context(tc.tile_pool(name="wp", bufs=1))
    inp = ctx.enter_context(tc.tile_pool(name="inp", bufs=3))
    act = ctx.enter_context(tc.tile_pool(name="act", bufs=1))
    # PSUM banks: gpp(proj q/k + v-direct) 2 + sc 3 + msg 1 + mlp(den,y,z) 2 = 8
    gpp = ctx.enter_context(tc.tile_pool(name="gpp", bufs=2, space="PSUM"))
    scp = ctx.enter_context(tc.tile_pool(name="scp", bufs=3, space="PSUM"))
    mdp = ctx.enter_context(tc.tile_pool(name="mdp", bufs=1, space="PSUM"))
    mlp = ctx.enter_context(tc.tile_pool(name="mlp", bufs=2, space="PSUM"))

    # --- packed weights: one DMA; everything else is AP slices of it ---
    wt = wp.tile([128, WTOT], BF16, tag="wt")
    nc.scalar.dma_start(out=wt[:], in_=aps['wpack'][:])

    def wslice(off, width):
        return wt[:, off:off + width]

    def pbcol(i):
        return wt[:, OFF_PB + 2 * i:OFF_PB + 2 * i + 2].bitcast(F32)

    def maskcol(m):
        return wt[:, OFF_MASK + 2 * m:OFF_MASK + 2 * m + 2].bitcast(F32)

    # --- input tiles: 3-buffer rotation, 1-slot DMA lookahead ---
    dst_t = [None] * slots
    src_t = [None] * slots

    def load_slot(s):
        dt_ = inp.tile([128, 2, ND[s]], BF16, tag="dst", name=f"dst{s}")
        st_ = inp.tile([128, 2, NS[s]], BF16, tag="src", name=f"src{s}")
        # slots 1-2 (the only loads hoistable ahead of the first matmuls)
        # ride the gpsimd queue so the sync-queue clock stays at slot 0
        eng = nc.gpsimd if s in (1, 2) else nc.sync
        eng.dma_start(out=dt_[:], in_=_dram3(aps['dstT'], int(doff[s]), int(ND[s]), plan['WD']))
        eng.dma_start(out=st_[:], in_=_dram3(aps['srcT'], int(soff[s]), int(NS[s]), plan['WS']))
        dst_t[s] = dt_
        src_t[s] = st_

    load_slot(0)

    # banded V slots, zero-filled once via the gpsimd (SWDGE) queue: the
    # first matmuls wait on the sync/scalar queue clocks, so zero-fill and
    # early prefetch must stay off those queues
    v_slots = []
    for i in range(3):
        vs = act.tile([128, CHmax, HEADS, C], BF16, tag=f"Vs{i}", name=f"Vs{i}")
        nc.gpsimd.dma_start(
            out=vs[:],
            in_=aps['vzero'].rearrange("p (a h c) -> p a h c", a=4, h=HEADS)[:, :CHmax])
        v_slots.append(vs)

    for s in range(slots):
        nd_, ns_, ch = int(ND[s]), int(NS[s]), int(CH[s])
        dt_, st_ = dst_t[s], src_t[s]

        # ---------- projections ----------
        ps_q = gpp.tile([128, NDmax], F32, tag="gpp", name=f"psq{s}")
        for a in range(2):
            nc.tensor.matmul(ps_q[:, :nd_], wslice(OFF_WQ + a * 128, 128), dt_[:, a, :],
                             start=(a == 0), stop=(a == 1))
        q_t = act.tile([128, NDmax], BF16, tag="q", name=f"q{s}", bufs=3)
        nc.vector.tensor_scalar_add(q_t[:, :nd_], ps_q[:, :nd_], pbcol(0))

        ps_k = gpp.tile([128, NSmax], F32, tag="gpp", name=f"psk{s}")
        for a in range(2):
            nc.tensor.matmul(ps_k[:, :ns_], wslice(OFF_WK + a * 128, 128), st_[:, a, :],
                             start=(a == 0), stop=(a == 1))
        k_t = act.tile([128, NSmax], BF16, tag="k", name=f"k{s}", bufs=3)
        nc.vector.tensor_scalar_add(k_t[:, :ns_], ps_k[:, :ns_], pbcol(1))

        # ---------- V direct (natural [tok, chan]); bias folded into MLP1 ----------
        ps_vd = gpp.tile([128, CHmax, 128], F32, tag="gpp", name=f"psvd{s}")
        for j in range(ch):
            for a in range(2):
                nc.tensor.matmul(ps_vd[:, j, :],
                                 st_[:, a, j * 128:(j + 1) * 128],
                                 wslice(OFF_WV + a * 128, 128),
                                 start=(a == 0), stop=(a == 1))
        v_sb = v_slots[s % 3]
        vdst = bass.AP(tensor=v_sb.tensor, offset=v_sb.offset,
                       ap=[v_sb.ap[0]] + [[HEADS * C, ch], [C + DH, HEADS], [1, DH]])
        vsrc = bass.AP(tensor=ps_vd.tensor, offset=ps_vd.offset,
                       ap=[ps_vd.ap[0]] + [[128, ch], [DH, HEADS], [1, DH]])
        nc.vector.tensor_copy(vdst, vsrc)

        if s + 1 < slots:
            load_slot(s + 1)

        # ---------- attention ----------
        ps_msg = mdp.tile([128, NDmax], F32, tag="msg", name=f"psmsg{s}")
        e0 = [None] * HEADS     # head -> first chunk's e tile
        e_run = [None] * HEADS  # head -> chunk-sum accumulator
        for j in range(ch):
            for h in range(HEADS):
                ps_sc = scp.tile([128, 512], F32, tag="sc", name=f"pssc{s}_{j}_{h}")
                nc.tensor.matmul(
                    ps_sc[:, :nd_],
                    k_t[32 * h:32 * h + 32, j * 128:(j + 1) * 128],
                    q_t[32 * h:32 * h + 32, :nd_],
                    start=True, stop=True, tile_position=(32 * h, 0))
                e_t = act.tile([128, NDmax], BF16, tag="E", name=f"E{s}_{j}_{h}", bufs=8)
                if plan['maskfull'][moff[s] + j]:
                    nc.scalar.activation(e_t[:, :nd_], ps_sc[:, :nd_],
                                         mybir.ActivationFunctionType.Exp)
                else:
                    nc.scalar.activation(e_t[:, :nd_], ps_sc[:, :nd_],
                                         mybir.ActivationFunctionType.Exp,
                                         bias=maskcol(moff[s] + j))
                nc.tensor.matmul(
                    ps_msg[:, :nd_],
                    v_sb[:, j, h, :],
                    e_t[:, :nd_],
                    start=(j == 0 and h == 0), stop=(j == ch - 1 and h == 3))
                # denominator: sum e over src chunks on DVE/GpSimd, one
                # matmul per head at the end
                eng = nc.vector if h < 2 else nc.gpsimd
                if ch == 1:
                    e_run[h] = e_t
                elif j == 0:
                    e0[h] = e_t
                else:
                    er = act.tile([128, NDmax], BF16, tag=f"er{h}_{j % 2}",
                                  name=f"er{s}_{h}_{j}", bufs=2)
                    eng.tensor_add(er[:, :nd_], e_run[h][:, :nd_] if j > 1 else e0[h][:, :nd_],
                                   e_t[:, :nd_])
                    e_run[h] = er
        # last slot: split norm+MLP into dst halves so the tail pipeline
        # overlaps (no next slot to hide the recip->mul->MLP chain)
        halves = [(0, nd_)]
        ps_den = mlp.tile([128, NDmax], F32, tag="mlp", name=f"psden{s}")
        for h in range(HEADS):
            nc.tensor.matmul(
                ps_den[:, :nd_],
                wslice(OFF_ONES + h * 128, 128),
                e_run[h][:, :nd_],
                start=(h == 0), stop=(h == 3))
        r_sb = act.tile([128, NDmax], F32, tag="rsb", name=f"rsb{s}", bufs=2)
        msgn = act.tile([128, NDmax], BF16, tag="msgn", name=f"msgn{s}", bufs=2)
        out_sb = act.tile([128, 2, NDmax], BF16, tag="out", name=f"out{s}", bufs=2)
        ps_y = [mlp.tile([128, NDmax], F32, tag="mlp", name=f"psy{s}_{o}")
                for o in range(2)]
        ps_z = [mlp.tile([128, NDmax], F32, tag="mlp", name=f"psz{s}_{o}")
                for o in range(2)]
        y1 = [act.tile([128, NDmax], BF16, tag=f"y1_{o}", name=f"y1_{s}_{o}", bufs=2)
              for o in range(2)]
        for lo, hi in halves:
            w = hi - lo
            nc.vector.reciprocal_approx_fast(r_sb[:, lo:hi], ps_den[:, lo:hi])
            nc.vector.tensor_mul(msgn[:, lo:hi], ps_msg[:, lo:hi], r_sb[:, lo:hi])
            for o in range(2):
                rhs_list = [dt_[:, 0, lo:hi], dt_[:, 1, lo:hi], msgn[:, lo:hi]]
                for kk in range(3):
                    nc.tensor.matmul(ps_y[o][:, lo:hi],
                                     wslice(OFF_W1 + kk * 256 + o * 128, 128),
                                     rhs_list[kk], start=(kk == 0), stop=(kk == 2))
                # relu on ACT (same table-set as Exp): frees DVE queue time
                nc.scalar.activation(y1[o][:, lo:hi], ps_y[o][:, lo:hi],
                                     mybir.ActivationFunctionType.Relu,
                                     bias=pbcol(2 + o))
            for o in range(2):
                for kk in range(2):
                    nc.tensor.matmul(ps_z[o][:, lo:hi],
                                     wslice(OFF_W2 + kk * 256 + o * 128, 128),
                                     y1[kk][:, lo:hi], start=(kk == 0), stop=(kk == 1))
                # out = (W2@y1 + b2) + dst residual, fused on DVE
                nc.vector.scalar_tensor_tensor(
                    out_sb[:, o, lo:hi], ps_z[o][:, lo:hi], pbcol(4 + o),
                    dt_[:, o, lo:hi], op0=mybir.AluOpType.add,
                    op1=mybir.AluOpType.add)
        nc.sync.dma_start(out=_dram3(aps['outT'], int(doff[s]), nd_, plan['WD']),
                          in_=out_sb[:, :, :nd_])


def build_nc(plan=None):
    if plan is None:
        plan = _PLAN
    nc = bacc.Bacc("TRN2", target_bir_lowering=False, debug=False,
                   enable_asserts=True, num_devices=NCORES)
    aps = declare_tensors(nc, plan)
    with tile.TileContext(nc) as tc:
        with ExitStack() as ctx:
            build_body(ctx, tc, aps, plan)
    nc.compile()
    return nc


def in_map(core, shared=None):
    return dict(dstT=core['dstT'], srcT=core['srcT'], wpack=core['wpack'],
                vzero=np.zeros((128, 4 * HEADS * C), BF))


def assemble(outTs, meta):
    nd = meta['nd']
    doff_g = meta['doff_g']
    plan = meta['plan']
    out = np.empty((int(nd.sum()), H), np.float32)
    for c in range(NCORES):
        for j in range(plan['slots']):
            g = plan['assign'][c, j]
            sl = outTs[c][:, plan['doff'][j]: plan['doff'][j] + nd[g]]
            out[doff_g[g]:doff_g[g] + nd[g]] = sl.T.astype(np.float32)
    return out


def kernel(**inputs):
    cores, meta = host_prep(inputs)
    nc = build_nc(meta['plan'])
    in_maps = [in_map(cores[c]) for c in range(NCORES)]
    res = run_bass_kernel_spmd(nc, in_maps, core_ids=list(range(NCORES)))
    outTs = [np.asarray(res.results[c]["outT"]) for c in range(NCORES)]
    return assemble(outTs, meta)


# revision 30
# speedup vs baseline: 1.0419x; 1.0419x over previous
"""Trainium2 Bass kernel for nn_CrossAttentionLayer (ragged cross-attention + MLP).

Sharding: 64 ragged segments -> 8 cores x 8 slots. Segments are sorted by
(src-chunk count, dst count), dealt into slots of 8, then hill-climbed so
segments sharing a slot have similar sizes; each slot is trimmed to the max
dst count (ND) / src count (NS, rounded to 128) over its 8 segments, so all
cores run one SPMD program.

All matmul operands are bf16 (fp32 PSUM accumulation). Activations stay
channel-major [chan, tok]; softmax runs in scoresT orientation [src, dst]
with the src-padding mask applied as a per-partition bias on the exp. The
denominator is computed by summing e over src chunks on DVE/GpSimd, then one
banded-ones matmul per head replicates each head's denominator across its 32
partitions; normalization is reciprocal_approx_fast + multiply. V is
produced directly in natural [tok, chan] orientation (src chunks as
stationary); its bias and the merge conv + BN are folded into the MLP
weights on the host. The dst residual is added via an identity matmul
accumulated into the MLP2 PSUM group.

All weights/biases/masks ship in one packed bf16 DMA (f32 parts bitcast);
per-slot inputs/outputs move as single 3D-AP DMAs with a 3-buffer rotation
and 2-slot prefetch.
"""
import math
import sys
from contextlib import ExitStack

import numpy as np
import ml_dtypes

try:
    import concourse.bass as bass
except ImportError:
    sys.path.insert(0, "/opt/trn_rl_repo")
    import concourse.bass as bass

import concourse.tile as tile
from concourse import bacc, mybir
from concourse.bass_utils import run_bass_kernel_spmd

F32 = mybir.dt.float32
BF16 = mybir.dt.bfloat16
BF = ml_dtypes.bfloat16

H = 256          # h_dim
C = 128          # h_div
HEADS = 4
DH = 32
NCORES = 8
MASK_NEG = -20000.0

# packed-weights column offsets (bf16 elements per partition)
OFF_WQ = 0
OFF_WK = 256
OFF_WV = 512
OFF_W1 = 768
OFF_W2 = 1536
OFF_ONES = 2048
OFF_EYE = 2560
OFF_PB = 2688    # 6 f32 = 12 bf16
OFF_MASK = 2700  # WM f32 = 2*WM bf16

# Filled by host_prep; read by build_nc/in_map (same process).
_PLAN = {}


def _make_plan(nd, ns):
    """Assign 64 segments to (core, slot); compute per-slot widths."""
    B = len(nd)
    slots = B // NCORES
    chunks_of = np.ceil(ns / 128).astype(int)
    order = list(np.lexsort((-nd, -chunks_of)))

    def slot_cost(grp):
        ndm = -(-max(nd[g] for g in grp) // 4) * 4
        chm = max(chunks_of[g] for g in grp)
        return ndm * (14 + 8 * chm + 4) + 2 * 128 * chm + 256 * chm

    rng = np.random.default_rng(0)
    groups = [order[j * NCORES:(j + 1) * NCORES] for j in range(slots)]
    costs = [slot_cost(g) for g in groups]
    for _ in range(30000):
        j1, j2 = rng.integers(0, slots, 2)
        if j1 == j2:
            continue
        i1, i2 = rng.integers(0, NCORES, 2)
        g1, g2 = groups[j1][i1], groups[j2][i2]
        groups[j1][i1], groups[j2][i2] = g2, g1
        c1, c2 = slot_cost(groups[j1]), slot_cost(groups[j2])
        if c1 + c2 < costs[j1] + costs[j2]:
            costs[j1], costs[j2] = c1, c2
        else:
            groups[j1][i1], groups[j2][i2] = g1, g2
    # big slots first so the tail slot is the cheapest
    sidx = sorted(range(slots), key=lambda j: -costs[j])
    groups = [groups[j] for j in sidx]

    assign = np.empty((NCORES, slots), dtype=int)   # (core, slot) -> segment
    ND = np.empty(slots, dtype=int)
    NS = np.empty(slots, dtype=int)
    for j in range(slots):
        grp = groups[j]
        for c in range(NCORES):
            assign[c, j] = grp[c]
        ND[j] = int(-(-max(nd[g] for g in grp) // 4) * 4)       # mult of 4
        NS[j] = int(-(-max(ns[g] for g in grp) // 128) * 128)   # mult of 128
    CH = (NS // 128).astype(int)
    return dict(slots=slots, assign=assign, ND=ND, NS=NS, CH=CH,
                doff=np.concatenate([[0], np.cumsum(ND)[:-1]]),
                soff=np.concatenate([[0], np.cumsum(NS)[:-1]]),
                moff=np.concatenate([[0], np.cumsum(CH)[:-1]]),
                WD=int(ND.sum()), WS=int(NS.sum()), WM=int(CH.sum()))


def host_prep(inputs):
    src_h = np.asarray(inputs['src_h'], np.float32)
    dst_h = np.asarray(inputs['dst_h'], np.float32)
    ns = np.asarray(inputs['src_num_verts']).astype(np.int64)
    nd = np.asarray(inputs['dst_num_verts']).astype(np.int64)
    soff_g = np.concatenate([[0], np.cumsum(ns)[:-1]])
    doff_g = np.concatenate([[0], np.cumsum(nd)[:-1]])

    plan = _make_plan(nd, ns)
    global _PLAN
    _PLAN = plan
    slots = plan['slots']

    perm = np.empty(C, np.int64)
    for chat in range(C):
        h, d = divmod(chat, DH)
        perm[chat] = d * HEADS + h
    s = 1.0 / math.sqrt(DH)

    f32 = lambda k: np.asarray(inputs[k], np.float32)
    Wq, bq = f32('Wq'), f32('bq')
    Wk, bk = f32('Wk'), f32('bk')
    Wv, bv = f32('Wv'), f32('bv')
    Wm, bm = f32('Wm'), f32('bm')
    W1, b1 = f32('W1'), f32('b1')
    W2, b2 = f32('W2'), f32('b2')
    g1, be1, rm1, rv1 = f32('g1'), f32('be1'), f32('rm1'), f32('rv1')
    g2, be2, rm2, rv2 = f32('g2'), f32('be2'), f32('rm2'), f32('rv2')

    WqT = (Wq[perm] * s).T                                # [256,128]
    bq_s = bq[perm] * s
    WkT = Wk[perm].T
    WvT = Wv[perm].T
    bv_r = bv[perm]
    Wm_p = Wm[:, perm]
    a1 = g1 / np.sqrt(rv1 + 1e-5)
    W1_f = W1 * a1[:, None]
    b1_f = b1 * a1 + be1 - rm1 * a1
    a2 = g2 / np.sqrt(rv2 + 1e-5)
    W2_f = W2 * a2[:, None]
    b2_f = b2 * a2 + be2 - rm2 * a2
    W1m_p = W1_f[:, H:] @ Wm_p
    # V bias folded all the way into the MLP1 bias: msg enters MLP1 as
    # msg/den (no bias), and W1m_p @ bv_perm is a constant.
    b1_p = b1_f + W1_f[:, H:] @ bm + W1m_p @ bv_r
    W1T = np.concatenate([W1_f[:, :H], W1m_p], axis=1).T  # [384,256]
    W2T = W2_f.T                                          # [256,256]

    pbias = np.zeros((128, 6), np.float32)
    pbias[:, 0] = bq_s
    pbias[:, 1] = bk[perm]
    pbias[:, 2] = b1_p[:128]
    pbias[:, 3] = b1_p[128:]
    pbias[:, 4] = b2_f[:128]
    pbias[:, 5] = b2_f[128:]

    onespad = np.zeros((128, HEADS, C), BF)
    for h in range(HEADS):
        onespad[:, h, h * DH:(h + 1) * DH] = 1.0

    WM = plan['WM']
    WTOT = OFF_MASK + 2 * WM

    def pack_weights(maskb):
        wpk = np.zeros((128, WTOT), BF)
        wpk[:, OFF_WQ:OFF_WQ + 256] = WqT.reshape(2, 128, 128).transpose(1, 0, 2).reshape(128, 256).astype(BF)
        wpk[:, OFF_WK:OFF_WK + 256] = WkT.reshape(2, 128, 128).transpose(1, 0, 2).reshape(128, 256).astype(BF)
        wpk[:, OFF_WV:OFF_WV + 256] = WvT.reshape(2, 128, 128).transpose(1, 0, 2).reshape(128, 256).astype(BF)
        wpk[:, OFF_W1:OFF_W1 + 768] = W1T.reshape(3, 128, 256).transpose(1, 0, 2).reshape(128, 768).astype(BF)
        wpk[:, OFF_W2:OFF_W2 + 512] = W2T.reshape(2, 128, 256).transpose(1, 0, 2).reshape(128, 512).astype(BF)
        wpk[:, OFF_ONES:OFF_ONES + 512] = onespad.reshape(128, 512)
        wpk[:, OFF_EYE:OFF_EYE + 128] = np.eye(128, dtype=BF)
        wpk[:, OFF_PB:OFF_PB + 12] = pbias.view(BF)
        wpk[:, OFF_MASK:OFF_MASK + 2 * WM] = maskb.view(BF)
        return wpk

    ND, NS, CH = plan['ND'], plan['NS'], plan['CH']
    doff, soff, moff = plan['doff'], plan['soff'], plan['moff']
    # chunk fully valid on every core -> exp needs no mask bias
    full = np.ones(plan['WM'], bool)
    for j in range(slots):
        for jj in range(CH[j]):
            full[moff[j] + jj] = all(
                int(ns[plan['assign'][c, j]]) >= 128 * (jj + 1) for c in range(NCORES))
    plan['maskfull'] = full
    cores = []
    for c in range(NCORES):
        dstT = np.zeros((H, plan['WD']), BF)
        srcT = np.zeros((H, plan['WS']), BF)
        maskb = np.full((128, WM), MASK_NEG, np.float32)
        for j in range(slots):
            g = plan['assign'][c, j]
            dstT[:, doff[j]:doff[j] + nd[g]] = dst_h[doff_g[g]:doff_g[g] + nd[g]].T.astype(BF)
            srcT[:, soff[j]:soff[j] + ns[g]] = src_h[soff_g[g]:soff_g[g] + ns[g]].T.astype(BF)
            for jj in range(CH[j]):
                valid = max(0, min(128, int(ns[g]) - jj * 128))
                maskb[:valid, moff[j] + jj] = 0.0
        cores.append(dict(dstT=dstT, srcT=srcT, wpack=pack_weights(maskb)))

    meta = dict(nd=nd, doff_g=doff_g, plan=plan)
    return cores, meta


def declare_tensors(nc, plan):
    WTOT = OFF_MASK + 2 * plan['WM']
    aps = {}
    aps['dstT'] = nc.dram_tensor("dstT", [H, plan['WD']], BF16, kind="ExternalInput").ap()
    aps['srcT'] = nc.dram_tensor("srcT", [H, plan['WS']], BF16, kind="ExternalInput").ap()
    aps['wpack'] = nc.dram_tensor("wpack", [128, WTOT], BF16, kind="ExternalInput").ap()
    aps['outT'] = nc.dram_tensor("outT", [H, plan['WD']], BF16, kind="ExternalOutput").ap()
    return aps


def _dram3(ap, col0, width, total_w):
    """AP over a [256, total_w] dram tensor: [p=row%128, a=row//128, w]."""
    return bass.AP(tensor=ap.tensor, offset=col0,
                   ap=[[total_w, 128], [128 * total_w, 2], [1, width]])


def build_body(ctx: ExitStack, tc: tile.TileContext, aps, plan):
    nc = tc.nc
    slots = plan['slots']
    ND, NS, CH = plan['ND'], plan['NS'], plan['CH']
    doff, soff, moff = plan['doff'], plan['soff'], plan['moff']
    NDmax = int(ND.max())
    NSmax = int(NS.max())
    CHmax = int(CH.max())
    WTOT = OFF_MASK + 2 * plan['WM']

    wp = ctx.enter_context(tc.tile_pool(name="wp", bufs=1))
    inp = ctx.enter_context(tc.tile_pool(name="inp", bufs=3))
    act = ctx.enter_context(tc.tile_pool(name="act", bufs=1))
    # PSUM banks: gpp(proj q/k + v-direct) 2 + sc 3 + msg 1 + mlp(den,y,z) 2 = 8
    gpp = ctx.enter_context(tc.tile_pool(name="gpp", bufs=2, space="PSUM"))
    scp = ctx.enter_context(tc.tile_pool(name="scp", bufs=3, space="PSUM"))
    mdp = ctx.enter_context(tc.tile_pool(name="mdp", bufs=1, space="PSUM"))
    mlp = ctx.enter_context(tc.tile_pool(name="mlp", bufs=2, space="PSUM"))

    # --- packed weights: one DMA; everything else is AP slices of it ---
    wt = wp.tile([128, WTOT], BF16, tag="wt")
    nc.scalar.dma_start(out=wt[:], in_=aps['wpack'][:])

    def wslice(off, width):
        return wt[:, off:off + width]

    def pbcol(i):
        return wt[:, OFF_PB + 2 * i:OFF_PB + 2 * i + 2].bitcast(F32)

    def maskcol(m):
        return wt[:, OFF_MASK + 2 * m:OFF_MASK + 2 * m + 2].bitcast(F32)

    # --- input tiles: 3-buffer rotation, 1-slot DMA lookahead ---
    dst_t = [None] * slots
    src_t = [None] * slots

    def load_slot(s):
        dt_ = inp.tile([128, 2, ND[s]], BF16, tag="dst", name=f"dst{s}")
        st_ = inp.tile([128, 2, NS[s]], BF16, tag="src", name=f"src{s}")
        nc.sync.dma_start(out=dt_[:], in_=_dram3(aps['dstT'], int(doff[s]), int(ND[s]), plan['WD']))
        nc.sync.dma_start(out=st_[:], in_=_dram3(aps['srcT'], int(soff[s]), int(NS[s]), plan['WS']))
        dst_t[s] = dt_
        src_t[s] = st_

    load_slot(0)

    # banded V slots, zero-filled once (gpsimd memset: DVE is needed
    # immediately for the q/k bias adds)
    v_slots = []
    for i in range(3):
        vs = act.tile([128, CHmax, HEADS, C], BF16, tag=f"Vs{i}", name=f"Vs{i}")
        nc.gpsimd.memset(vs[:], 0.0)
        v_slots.append(vs)

    for s in range(slots):
        nd_, ns_, ch = int(ND[s]), int(NS[s]), int(CH[s])
        dt_, st_ = dst_t[s], src_t[s]

        # ---------- projections ----------
        ps_q = gpp.tile([128, NDmax], F32, tag="gpp", name=f"psq{s}")
        for a in range(2):
            nc.tensor.matmul(ps_q[:, :nd_], wslice(OFF_WQ + a * 128, 128), dt_[:, a, :],
                             start=(a == 0), stop=(a == 1))
        q_t = act.tile([128, NDmax], BF16, tag="q", name=f"q{s}", bufs=3)
        nc.vector.tensor_scalar_add(q_t[:, :nd_], ps_q[:, :nd_], pbcol(0))

        ps_k = gpp.tile([128, NSmax], F32, tag="gpp", name=f"psk{s}")
        for a in range(2):
            nc.tensor.matmul(ps_k[:, :ns_], wslice(OFF_WK + a * 128, 128), st_[:, a, :],
                             start=(a == 0), stop=(a == 1))
        k_t = act.tile([128, NSmax], BF16, tag="k", name=f"k{s}", bufs=3)
        nc.vector.tensor_scalar_add(k_t[:, :ns_], ps_k[:, :ns_], pbcol(1))

        # ---------- V direct (natural [tok, chan]); bias folded into MLP1 ----------
        ps_vd = gpp.tile([128, CHmax, 128], F32, tag="gpp", name=f"psvd{s}")
        for j in range(ch):
            for a in range(2):
                nc.tensor.matmul(ps_vd[:, j, :],
                                 st_[:, a, j * 128:(j + 1) * 128],
                                 wslice(OFF_WV + a * 128, 128),
                                 start=(a == 0), stop=(a == 1))
        v_sb = v_slots[s % 3]
        vdst = bass.AP(tensor=v_sb.tensor, offset=v_sb.offset,
                       ap=[v_sb.ap[0]] + [[HEADS * C, ch], [C + DH, HEADS], [1, DH]])
        vsrc = bass.AP(tensor=ps_vd.tensor, offset=ps_vd.offset,
                       ap=[ps_vd.ap[0]] + [[128, ch], [DH, HEADS], [1, DH]])
        nc.vector.tensor_copy(vdst, vsrc)

        if s + 1 < slots:
            load_slot(s + 1)

        # ---------- attention ----------
        ps_msg = mdp.tile([128, NDmax], F32, tag="msg", name=f"psmsg{s}")
        e0 = [None] * HEADS     # head -> first chunk's e tile
        e_run = [None] * HEADS  # head -> chunk-sum accumulator
        for j in range(ch):
            for h in range(HEADS):
                ps_sc = scp.tile([128, 512], F32, tag="sc", name=f"pssc{s}_{j}_{h}")
                nc.tensor.matmul(
                    ps_sc[:, :nd_],
                    k_t[32 * h:32 * h + 32, j * 128:(j + 1) * 128],
                    q_t[32 * h:32 * h + 32, :nd_],
                    start=True, stop=True, tile_position=(32 * h, 0))
                e_t = act.tile([128, NDmax], BF16, tag="E", name=f"E{s}_{j}_{h}", bufs=8)
                if plan['maskfull'][moff[s] + j]:
                    nc.scalar.activation(e_t[:, :nd_], ps_sc[:, :nd_],
                                         mybir.ActivationFunctionType.Exp)
                else:
                    nc.scalar.activation(e_t[:, :nd_], ps_sc[:, :nd_],
                                         mybir.ActivationFunctionType.Exp,
                                         bias=maskcol(moff[s] + j))
                nc.tensor.matmul(
                    ps_msg[:, :nd_],
                    v_sb[:, j, h, :],
                    e_t[:, :nd_],
                    start=(j == 0 and h == 0), stop=(j == ch - 1 and h == 3))
                # denominator: sum e over src chunks on DVE/GpSimd, one
                # matmul per head at the end
                eng = nc.vector if h < 2 else nc.gpsimd
                if ch == 1:
                    e_run[h] = e_t
                elif j == 0:
                    e0[h] = e_t
                else:
                    er = act.tile([128, NDmax], BF16, tag=f"er{h}_{j % 2}",
                                  name=f"er{s}_{h}_{j}", bufs=2)
                    eng.tensor_add(er[:, :nd_], e_run[h][:, :nd_] if j > 1 else e0[h][:, :nd_],
                                   e_t[:, :nd_])
                    e_run[h] = er
        # last slot: split norm+MLP into dst halves so the tail pipeline
        # overlaps (no next slot to hide the recip->mul->MLP chain)
        halves = [(0, nd_)]
        ps_den = mlp.tile([128, NDmax], F32, tag="mlp", name=f"psden{s}")
        for h in range(HEADS):
            nc.tensor.matmul(
                ps_den[:, :nd_],
                wslice(OFF_ONES + h * 128, 128),
                e_run[h][:, :nd_],
                start=(h == 0), stop=(h == 3))
        r_sb = act.tile([128, NDmax], F32, tag="rsb", name=f"rsb{s}", bufs=2)
        msgn = act.tile([128, NDmax], BF16, tag="msgn", name=f"msgn{s}", bufs=2)
        out_sb = act.tile([128, 2, NDmax], BF16, tag="out", name=f"out{s}", bufs=2)
        ps_y = [mlp.tile([128, NDmax], F32, tag="mlp", name=f"psy{s}_{o}")
                for o in range(2)]
        ps_z = [mlp.tile([128, NDmax], F32, tag="mlp", name=f"psz{s}_{o}")
                for o in range(2)]
        y1 = [act.tile([128, NDmax], BF16, tag=f"y1_{o}", name=f"y1_{s}_{o}", bufs=2)
              for o in range(2)]
        for lo, hi in halves:
            w = hi - lo
            nc.vector.reciprocal_approx_fast(r_sb[:, lo:hi], ps_den[:, lo:hi])
            nc.vector.tensor_mul(msgn[:, lo:hi], ps_msg[:, lo:hi], r_sb[:, lo:hi])
            for o in range(2):
                rhs_list = [dt_[:, 0, lo:hi], dt_[:, 1, lo:hi], msgn[:, lo:hi]]
                for kk in range(3):
                    nc.tensor.matmul(ps_y[o][:, lo:hi],
                                     wslice(OFF_W1 + kk * 256 + o * 128, 128),
                                     rhs_list[kk], start=(kk == 0), stop=(kk == 2))
                # relu on ACT (same table-set as Exp): frees DVE queue time
                nc.scalar.activation(y1[o][:, lo:hi], ps_y[o][:, lo:hi],
                                     mybir.ActivationFunctionType.Relu,
                                     bias=pbcol(2 + o))
            for o in range(2):
                for kk in range(2):
                    nc.tensor.matmul(ps_z[o][:, lo:hi],
                                     wslice(OFF_W2 + kk * 256 + o * 128, 128),
                                     y1[kk][:, lo:hi], start=(kk == 0), stop=(kk == 1))
                # out = (W2@y1 + b2) + dst residual, fused on DVE
                nc.vector.scalar_tensor_tensor(
                    out_sb[:, o, lo:hi], ps_z[o][:, lo:hi], pbcol(4 + o),
                    dt_[:, o, lo:hi], op0=mybir.AluOpType.add,
                    op1=mybir.AluOpType.add)
        nc.sync.dma_start(out=_dram3(aps['outT'], int(doff[s]), nd_, plan['WD']),
                          in_=out_sb[:, :, :nd_])


def build_nc(plan=None):
    if plan is None:
        plan = _PLAN
    nc = bacc.Bacc("TRN2", target_bir_lowering=False, debug=False,
                   enable_asserts=True, num_devices=NCORES)
    aps = declare_tensors(nc, plan)
    with tile.TileContext(nc) as tc:
        with ExitStack() as ctx:
            build_body(ctx, tc, aps, plan)
    nc.compile()
    return nc


def in_map(core, shared=None):
    return dict(dstT=core['dstT'], srcT=core['srcT'], wpack=core['wpack'])


def assemble(outTs, meta):
    nd = meta['nd']
    doff_g = meta['doff_g']
    plan = meta['plan']
    out = np.empty((int(nd.sum()), H), np.float32)
    for c in range(NCORES):
        for j in range(plan['slots']):
            g = plan['assign'][c, j]
            sl = outTs[c][:, plan['doff'][j]: plan['doff'][j] + nd[g]]
            out[doff_g[g]:doff_g[g] + nd[g]] = sl.T.astype(np.float32)
    return out


def kernel(**inputs):
    cores, meta = host_prep(inputs)
    nc = build_nc(meta['plan'])
    in_maps = [in_map(cores[c]) for c in range(NCORES)]
    res = run_bass_kernel_spmd(nc, in_maps, core_ids=list(range(NCORES)))
    outTs = [np.asarray(res.results[c]["outT"]) for c in range(NCORES)]
    return assemble(outTs, meta)
